# revision 1
# baseline (speedup 1.0000x reference)
"""Trainium2 Bass kernel for a dense transformer block (pre-LN, causal MHA + FFN).

Sharding: pure data-parallel over batch — 8 sequences -> 8 NeuronCores, no
collectives. Each core runs the full block on its [2048, 400] slice.

Per-core recipe (bf16 matmuls, f32 PSUM/residual/softmax-stats):
  h    = LN1(x)            -> bf16, PE-transposed into hT [400(c),2048(t)]
  qT   = bf16(0.1*Wq[h].T @ hT)   [100(d), 2048] per head
  kT   = bf16(Wk[h].T @ hT)
  v1   = bf16(hT.T @ Wv_all) rows + ones column  [2048(s), H, 102]
  attention per head over t-tiles of 512: scoresT = kT_chunk.T @ qT_tile
  ([s128, t512] PSUM), causal mask added on diagonal pairs, Exp on ACT ->
  probsT bf16 (directly in attn@V lhsT layout — no transposes); attn@V
  accumulates [t128, 102] where col 100 is the softmax denominator (ones
  column of v1); rows scaled by 1/denom at copy-out; transposed once into
  attn_oT [100(d), head, 2048].
  proj = sum_h attn_oT[h].T @ Wo[h] + residual into x (f32)
  LN2 -> h2T (reuses hT slot); FFN pipelined in 256-column slices:
  ffT = relu(W1.T @ h2T + b1) bf16 (transposed form), then
  fc2 rows = ffT.T @ W2 + residual + b2 -> out (f32).

All weight reshaping/casting is host-side numpy, shipped as ExternalInputs.
"""

import numpy as np
import ml_dtypes

import concourse.bass as bass
import concourse.mybir as mybir
import concourse.tile as tile
from concourse import bacc
from concourse.bass_utils import run_bass_kernel_spmd

BF16NP = ml_dtypes.bfloat16
BF16 = mybir.dt.bfloat16
F32 = mybir.dt.float32
AF = mybir.ActivationFunctionType
ALU = mybir.AluOpType

P = 128          # partitions
B = 8            # batch -> cores
T = 2048         # sequence length
C = 400          # embed dim
H = 4            # heads
D = 100          # head dim
DFF = 1600       # ffn hidden
NT = T // P      # 16 row tiles
NCC = C // D     # 4 contraction chunks of 100
WT = 512         # wide tile for qkv matmuls
NWT = T // WT    # 4
TJ = 512         # t-tile width for transposed attention scores
NTJ = T // TJ    # 4
SUB = TJ // P    # 4 t128 sub-blocks per score tile
FT = 256         # ffn column-slice width
NFT = T // FT    # 8
NFC = (DFF + P - 1) // P  # 13 f-chunks (12x128 + 64)
NEG = -1.0e30

LAST_RESULT = None  # BassKernelResults of the most recent run (for test.py)


def _fchunk(fc):
    return min(P, DFF - fc * P)


def build_block(loop_n=None, phases=("qkv", "attn", "proj", "ffn")):
    nc = bacc.Bacc("TRN2", target_bir_lowering=False, debug=False)

    x_d = nc.dram_tensor("x", [T, C], F32, kind="ExternalInput")
    wq_d = nc.dram_tensor("wqp", [D, H, NCC, P], BF16, kind="ExternalInput")
    wk_d = nc.dram_tensor("wkp", [D, H, NCC, P], BF16, kind="ExternalInput")
    wv_d = nc.dram_tensor("wvp", [D, NCC, C], BF16, kind="ExternalInput")
    wo_d = nc.dram_tensor("wop", [D, H, C], BF16, kind="ExternalInput")
    w1_d = nc.dram_tensor("w1p", [D, NCC, DFF], BF16, kind="ExternalInput")
    w2_d = nc.dram_tensor("w2p", [P, NFC, C], BF16, kind="ExternalInput")
    b1_d = nc.dram_tensor("b1p", [P, NFC], F32, kind="ExternalInput")
    bo_d = nc.dram_tensor("bop", [P, C], F32, kind="ExternalInput")
    b2_d = nc.dram_tensor("b2p", [P, C], F32, kind="ExternalInput")
    g1_d = nc.dram_tensor("g1p", [P, C], F32, kind="ExternalInput")
    be1_d = nc.dram_tensor("be1p", [P, C], F32, kind="ExternalInput")
    g2_d = nc.dram_tensor("g2p", [P, C], F32, kind="ExternalInput")
    be2_d = nc.dram_tensor("be2p", [P, C], F32, kind="ExternalInput")
    mask_d = nc.dram_tensor("maskp", [P, P], F32, kind="ExternalInput")
    id_d = nc.dram_tensor("identp", [P, P], BF16, kind="ExternalInput")
    out_d = nc.dram_tensor("out", [T, C], F32, kind="ExternalOutput")

    with tile.TileContext(nc) as tc:
        with (
            tc.tile_pool(name="consts", bufs=1) as consts,
            tc.tile_pool(name="persist", bufs=1) as persist,
            tc.tile_pool(name="qk", bufs=2) as qk_pool,
            tc.tile_pool(name="pr", bufs=2) as pr_pool,
            tc.tile_pool(name="fft", bufs=2) as fft_pool,
            tc.tile_pool(name="work", bufs=3) as work,
            tc.tile_pool(name="small", bufs=4) as small,
            tc.tile_pool(name="ps_mm", bufs=3, space="PSUM") as ps_mm,
            tc.tile_pool(name="ps_tr", bufs=1, space="PSUM") as ps_tr,
            tc.tile_pool(name="ps_av", bufs=2, space="PSUM") as ps_av,
            tc.tile_pool(name="ps_g", bufs=2, space="PSUM") as ps_g,
        ):
            # ---- constants into SBUF (one-time) ----
            def cload(tag, dram, shape, dtype, psz=P):
                t_ = consts.tile(shape, dtype, tag=tag)
                nc.sync.dma_start(t_[:psz], dram[:])
                return t_

            wq_sb = cload("wq", wq_d, [P, H, NCC, P], BF16, D)
            wk_sb = cload("wk", wk_d, [P, H, NCC, P], BF16, D)
            wv_sb = cload("wv", wv_d, [P, NCC, C], BF16, D)
            wo_sb = cload("wo", wo_d, [P, H, C], BF16, D)
            w1_sb = cload("w1", w1_d, [P, NCC, DFF], BF16, D)
            w2_sb = cload("w2", w2_d, [P, NFC, C], BF16)
            b1_sb = cload("b1", b1_d, [P, NFC], F32)
            bo_sb = cload("bo", bo_d, [P, C], F32)
            b2_sb = cload("b2", b2_d, [P, C], F32)
            g1_sb = cload("g1", g1_d, [P, C], F32)
            be1_sb = cload("be1", be1_d, [P, C], F32)
            g2_sb = cload("g2", g2_d, [P, C], F32)
            be2_sb = cload("be2", be2_d, [P, C], F32)
            mask_sb = cload("mask", mask_d, [P, P], F32)
            id_sb = cload("ident", id_d, [P, P], BF16)
            eps_sb = consts.tile([P, 1], F32, tag="eps")
            nc.vector.memset(eps_sb, 1e-5)

            def body():
                # ---- x into SBUF, tiled [128, 16, 400] ----
                x_sb = persist.tile([P, NT, C], F32, tag="x")
                nc.sync.dma_start(x_sb[:],
                                  x_d.rearrange("(n p) c -> p n c", p=P))

                hT_sb = persist.tile([P, NCC, T], BF16, tag="hT")
                v1_sb = persist.tile([P, NT, H, D + 2], BF16, tag="v")
                nc.vector.memset(v1_sb[:, :, :, D], 1.0)
                nc.vector.memset(v1_sb[:, :, :, D + 1], 0.0)
                ao_sb = persist.tile([P, H, T], BF16, tag="aoT")

                def layernorm(src3, g_sb, be_sb, dstT, tis):
                    """LN over row tiles src3[:, ti, :]; bf16 result
                    transposed into dstT[:D, cc, ti*P:(ti+1)*P].
                    Batches the sqrt/reciprocal across all tiles."""
                    n = len(tis)
                    mv = small.tile([P, n, 2], F32, tag="mv")
                    for k, ti in enumerate(tis):
                        stats = small.tile([P, 6], F32, tag="stats")
                        nc.vector.bn_stats(out=stats, in_=src3[:, ti, :])
                        nc.vector.bn_aggr(out=mv[:, k, :], in_=stats)
                    rstd = small.tile([P, n], F32, tag="rstd")
                    nc.scalar.activation(
                        out=rstd, in_=mv[:, :, 1], func=AF.Sqrt,
                        bias=eps_sb, scale=1.0)
                    nc.vector.reciprocal(out=rstd, in_=rstd)
                    for k, ti in enumerate(tis):
                        hrow = work.tile([P, C], F32, tag="hrow")
                        nc.vector.tensor_scalar(
                            out=hrow, in0=src3[:, ti, :],
                            scalar1=mv[:, k, 0:1], scalar2=rstd[:, k:k + 1],
                            op0=ALU.subtract, op1=ALU.mult)
                        nc.vector.tensor_mul(out=hrow, in0=hrow, in1=g_sb)
                        hbf = work.tile([P, C], BF16, tag="hbf")
                        nc.vector.tensor_add(out=hbf, in0=hrow, in1=be_sb)
                        for cc in range(NCC):
                            pt = ps_tr.tile([P, P], BF16, tag="tr")
                            nc.tensor.transpose(
                                pt[:D, :], hbf[:, cc * D:(cc + 1) * D], id_sb)
                            dst = dstT[:D, cc, ti * P:(ti + 1) * P]
                            if cc % 2 == 0:
                                nc.vector.tensor_copy(out=dst, in_=pt[:D, :])
                            else:
                                nc.scalar.copy(out=dst, in_=pt[:D, :])

                # ---- LN1 + transpose for all row tiles ----
                layernorm(x_sb, g1_sb, be1_sb, hT_sb, list(range(NT)))

                # ---- V rows (all heads) + ones column ----
                for ti in range(NT if "qkv" in phases else 0):
                    psv = ps_mm.tile([P, WT], F32, tag="mm")
                    for cc in range(NCC):
                        nc.tensor.matmul(
                            psv[:, :C],
                            lhsT=hT_sb[:D, cc, ti * P:(ti + 1) * P],
                            rhs=wv_sb[:D, cc, :],
                            start=(cc == 0), stop=(cc == NCC - 1))
                    nc.vector.tensor_copy(
                        out=v1_sb[:, ti, :, :D],
                        in_=psv[:, :C].rearrange("p (h d) -> p h d", h=H))

                # ---- per-head attention (transposed-score form) ----
                # attn@V is software-pipelined one score-tile behind the
                # scores/exp producer (carried across heads) so independent
                # matmuls hide the ACT exp latency on the in-order PE queue.
                def emit_attnv(pjT, h_, j):
                    for jj in range(SUB):
                        ti = SUB * j + jj
                        pso = ps_av.tile([P, P], F32, tag="av")
                        for si in range(ti + 1):
                            nc.tensor.matmul(
                                pso[:, :D + 2],
                                lhsT=pjT[:, si, jj * P:(jj + 1) * P],
                                rhs=v1_sb[:, si, h_, :],
                                start=(si == 0), stop=(si == ti))
                        rec = small.tile([P, 1], F32, tag="rec")
                        nc.vector.reciprocal(out=rec, in_=pso[:, D:D + 1])
                        arow = work.tile([P, D], BF16, tag="arow")
                        nc.vector.tensor_scalar_mul(
                            out=arow, in0=pso[:, :D], scalar1=rec)
                        pta = ps_tr.tile([P, P], BF16, tag="tr")
                        nc.tensor.transpose(pta[:D, :], arow, id_sb)
                        nc.vector.tensor_copy(
                            out=ao_sb[:D, h_, ti * P:(ti + 1) * P],
                            in_=pta[:D, :])

                pend_av = None
                for h in range(H if "qkv" in phases else 0):
                    qT = qk_pool.tile([P, T], BF16, tag="qT")
                    kT = qk_pool.tile([P, T], BF16, tag="kT")
                    for tt in range(NWT):
                        sl = slice(tt * WT, (tt + 1) * WT)
                        psq = ps_mm.tile([P, WT], F32, tag="mm")
                        for cc in range(NCC):
                            nc.tensor.matmul(
                                psq, lhsT=wq_sb[:D, h, cc, :],
                                rhs=hT_sb[:D, cc, sl],
                                start=(cc == 0), stop=(cc == NCC - 1))
                        if tt % 2 == 0:
                            nc.vector.tensor_scalar_mul(
                                out=qT[:D, sl], in0=psq[:D, :], scalar1=0.1)
                        else:
                            nc.scalar.mul(out=qT[:D, sl], in_=psq[:D, :],
                                          mul=0.1)
                        psk = ps_mm.tile([P, WT], F32, tag="mm")
                        for cc in range(NCC):
                            nc.tensor.matmul(
                                psk, lhsT=wk_sb[:D, h, cc, :],
                                rhs=hT_sb[:D, cc, sl],
                                start=(cc == 0), stop=(cc == NCC - 1))
                        if tt % 2 == 0:
                            nc.vector.tensor_copy(out=kT[:D, sl],
                                                  in_=psk[:D, :])
                        else:
                            nc.scalar.copy(out=kT[:D, sl], in_=psk[:D, :])

                    for j in range(NTJ if "attn" in phases else 0):
                        icnt = SUB * j + SUB
                        pjT = pr_pool.tile([P, NT, TJ], BF16, tag="probsT")
                        for i in range(icnt):
                            pss = ps_mm.tile([P, WT], F32, tag="mm")
                            nc.tensor.matmul(
                                pss[:, :TJ], lhsT=kT[:D, i * P:(i + 1) * P],
                                rhs=qT[:D, j * TJ:(j + 1) * TJ],
                                start=True, stop=True)
                            r = i - SUB * j
                            if r >= 0:
                                # only the diagonal t128 sub-block needs the
                                # causal mask: fully-masked sub-blocks (jj<r)
                                # land in probsT regions attn@V never reads.
                                nc.vector.tensor_add(
                                    out=pss[:, r * P:(r + 1) * P],
                                    in0=pss[:, r * P:(r + 1) * P],
                                    in1=mask_sb)
                            nc.scalar.activation(out=pjT[:, i, :],
                                                 in_=pss[:, :TJ], func=AF.Exp)
                        if pend_av is not None:
                            emit_attnv(*pend_av)
                        pend_av = (pjT, h, j)

                if pend_av is not None:
                    emit_attnv(*pend_av)

                # ---- output projection + residual ----
                for ti in range(NT if "proj" in phases else 0):
                    psp = ps_g.tile([P, WT], F32, tag="g")
                    for h in range(H):
                        nc.tensor.matmul(
                            psp[:, :C], lhsT=ao_sb[:D, h, ti * P:(ti + 1) * P],
                            rhs=wo_sb[:D, h, :],
                            start=(h == 0), stop=(h == H - 1))
                    nc.vector.tensor_add(out=x_sb[:, ti, :],
                                         in0=x_sb[:, ti, :], in1=psp[:, :C])
                    nc.gpsimd.tensor_add(out=x_sb[:, ti, :],
                                         in0=x_sb[:, ti, :], in1=bo_sb)

                # ---- FFN, pipelined in 256-column slices ----
                outr = out_d.rearrange("(n p) c -> p n c", p=P)
                if "ffn" in phases:
                    h2T = persist.tile([P, NCC, T], BF16, tag="hT")
                    layernorm(x_sb, g2_sb, be2_sb, h2T, list(range(NT)))
                    def emit_fc2(ffT, ft):
                        for tl in range(FT // P):
                            ti = ft * (FT // P) + tl
                            psg = ps_g.tile([P, WT], F32, tag="g")
                            for fc in range(NFC):
                                fsz = _fchunk(fc)
                                nc.tensor.matmul(
                                    psg[:, :C],
                                    lhsT=ffT[:fsz, fc, tl * P:(tl + 1) * P],
                                    rhs=w2_sb[:fsz, fc, :],
                                    start=(fc == 0), stop=(fc == NFC - 1))
                            orow = work.tile([P, C], F32, tag="orow")
                            nc.vector.tensor_add(out=orow, in0=psg[:, :C],
                                                 in1=x_sb[:, ti, :])
                            nc.gpsimd.tensor_add(out=orow, in0=orow,
                                                 in1=b2_sb)
                            nc.sync.dma_start(outr[:, ti, :], orow)

                    pend_fc2 = None
                    for ft in range(NFT):
                        sl = slice(ft * FT, (ft + 1) * FT)
                        ffT = fft_pool.tile([P, NFC, FT], BF16, tag="ffT")
                        for fc in range(NFC):
                            fsz = _fchunk(fc)
                            psf = ps_mm.tile([P, WT], F32, tag="mm")
                            for cc in range(NCC):
                                nc.tensor.matmul(
                                    psf[:fsz, :FT],
                                    lhsT=w1_sb[:D, cc, fc * P:fc * P + fsz],
                                    rhs=h2T[:D, cc, sl],
                                    start=(cc == 0), stop=(cc == NCC - 1))
                            if fc % 2 == 0:
                                nc.vector.tensor_scalar(
                                    out=ffT[:fsz, fc, :], in0=psf[:fsz, :FT],
                                    scalar1=b1_sb[:fsz, fc:fc + 1],
                                    scalar2=0.0, op0=ALU.add, op1=ALU.max)
                            else:
                                nc.scalar.activation(
                                    out=ffT[:fsz, fc, :], in_=psf[:fsz, :FT],
                                    func=AF.Relu,
                                    bias=b1_sb[:fsz, fc:fc + 1], scale=1.0)
                        if pend_fc2 is not None:
                            emit_fc2(*pend_fc2)
                        pend_fc2 = (ffT, ft)
                    emit_fc2(*pend_fc2)
                else:
                    zrow = work.tile([P, C], F32, tag="orow")
                    nc.vector.memset(zrow, 0.0)
                    for ti in range(NT):
                        nc.sync.dma_start(outr[:, ti, :], zrow)

            if loop_n is None:
                body()
            else:
                with tc.For_i(0, loop_n, 1):
                    body()

    nc.finalize()
    return nc


def prep_weights(Wq, Wk, Wv, Wo, bo, W1, b1, W2, b2,
                 ln1_g, ln1_b, ln2_g, ln2_b):
    """Host-side reshape/cast into the layouts the device program expects."""
    f32 = np.float32
    Wq = np.asarray(Wq, f32); Wk = np.asarray(Wk, f32)
    Wv = np.asarray(Wv, f32); Wo = np.asarray(Wo, f32)
    W1 = np.asarray(W1, f32); W2 = np.asarray(W2, f32)
    # [H, C, D] -> [c(100), H, cc, D->padded 128]
    wqp = np.zeros((D, H, NCC, P), BF16NP)
    wkp = np.zeros((D, H, NCC, P), BF16NP)
    wqp[:, :, :, :D] = Wq.reshape(H, NCC, D, D).transpose(2, 0, 1, 3
                                                          ).astype(BF16NP)
    wkp[:, :, :, :D] = Wk.reshape(H, NCC, D, D).transpose(2, 0, 1, 3
                                                          ).astype(BF16NP)
    # [H, C, D] -> [c(100), cc, H*D]
    wvp = (Wv.reshape(H, NCC, D, D).transpose(2, 1, 0, 3)
           .reshape(D, NCC, C).astype(BF16NP).copy())
    # [C, C] -> [c_in_head(100), H, C]
    wop = Wo.reshape(H, D, C).transpose(1, 0, 2).astype(BF16NP).copy()
    # [C, DFF] -> [c(100), cc, DFF]
    w1p = W1.reshape(NCC, D, DFF).transpose(1, 0, 2).astype(BF16NP).copy()
    # [DFF, C] -> [f_in_chunk(128), fc(13), C], zero-padded
    w2p = np.zeros((P, NFC, C), BF16NP)
    b1p = np.zeros((P, NFC), np.float32)
    for fc in range(NFC):
        fsz = _fchunk(fc)
        w2p[:fsz, fc, :] = W2[fc * P:fc * P + fsz, :].astype(BF16NP)
        b1p[:fsz, fc] = np.asarray(b1, f32)[fc * P:fc * P + fsz]
    tilep = lambda a: np.tile(np.asarray(a, f32).reshape(1, C), (P, 1)).copy()
    # transposed-score causal masks [s_local(128), r, t_local(TJ)]:
    # r = i - SUB*j; sub-block jj of the TJ cols is t128 index (SUB*j+jj).
    # masked (NEG) iff t < s: jj < r full, jj == r strict lower triangle.
    sl_ = np.arange(P)[:, None]
    tl_ = np.arange(P)[None, :]
    maskp = np.where(tl_ >= sl_, 0.0, NEG).astype(f32)   # 0 where t >= s
    ident = np.eye(P, dtype=BF16NP)
    return {
        "wqp": wqp, "wkp": wkp, "wvp": wvp, "wop": wop, "w1p": w1p,
        "w2p": w2p, "b1p": b1p, "bop": tilep(bo), "b2p": tilep(b2),
        "g1p": tilep(ln1_g), "be1p": tilep(ln1_b),
        "g2p": tilep(ln2_g), "be2p": tilep(ln2_b),
        "maskp": np.ascontiguousarray(maskp), "identp": ident,
    }


_CACHED_NC = None


def kernel(x, ln1_g, ln1_b, ln2_g, ln2_b, Wq, Wk, Wv, Wo, bo, W1, b1, W2, b2,
           trace=False):
    global _CACHED_NC, LAST_RESULT
    x = np.asarray(x, np.float32)
    assert x.shape == (B, T, C), x.shape
    wmap = prep_weights(Wq, Wk, Wv, Wo, bo, W1, b1, W2, b2,
                        ln1_g, ln1_b, ln2_g, ln2_b)
    if _CACHED_NC is None:
        _CACHED_NC = build_block()
    nc = _CACHED_NC
    in_maps = [dict(wmap, x=np.ascontiguousarray(x[c])) for c in range(B)]
    res = run_bass_kernel_spmd(nc, in_maps, core_ids=list(range(B)),
                               trace=trace)
    LAST_RESULT = res
    out = np.stack([res.results[c]["out"] for c in range(B)])
    return out.astype(np.float32)



# revision 27
# speedup vs baseline: 1.0285x; 1.0285x over previous
"""Trainium2 Bass kernel for a dense transformer block (pre-LN, causal MHA + FFN).

Sharding: pure data-parallel over batch — 8 sequences -> 8 NeuronCores, no
collectives. Each core runs the full block on its [2048, 400] slice.

Per-core recipe (bf16 matmuls, f32 PSUM/residual/softmax-stats):
  All LayerNorm gains/biases and projection biases are folded into the
  matmuls themselves: Wq/Wk/Wv/W1 rows are scaled by gamma host-side, and
  each of those matmuls gains an extra contraction row (the normalized
  activations carry a constant-1.0 row) whose weight row is the folded beta
  bias. bo rides the proj matmul through a constant-1.0 partition row of the
  attn-output tensor. So device-side LN is: bn_stats/bn_aggr -> sqrt ->
  reciprocal -> one tensor_scalar into bf16 rows.

  All [t,c]->[c,t] transposes (LN outputs and attention outputs) go through
  the DMA crossbar (dma_start_transpose, 16x128 xbar tiles) instead of the
  PE+PSUM+copy pipeline — they run on otherwise-idle DMA hardware.

  hT layout [128(c-chunk), 4, 2048(t)]: c chunks of 128 rows (last chunk 16
  rows + the ones row at partition 16).
  q/k per head: one [128,1024] 2-bank PSUM tile holds q|k halves, one DVE
  copy -> qk_sb [d, 2, t] bf16.
  attention per head over t-tiles of 512: scoresT = kT_chunk.T @ qT_tile;
  sub-diagonal score rows are computed/exp'd in PAIRS sharing a [128,1024]
  PSUM tile (one Exp per pair — ACT per-op overhead is ~240ns); diagonal
  rows are narrowed to the causally-live columns and the triangular mask is
  added by the PE as an extra accumulation step (maskT.T @ I). probsT bf16
  lands directly in attn@V lhsT layout. attn@V accumulates [t128, 102] per
  128-row block into a shared [128, 4, 102] one-bank PSUM tile per 512-wide
  j-tile (col 100 = softmax denominator via the ones column of v1); one
  batched reciprocal + one stride-0-broadcast tensor_tensor rescale, then
  one DMA-crossbar transpose into attn_oT [100(d), head, 2048].
  proj = sum_h attn_oT[h].T @ Wo[h] (+bo via ones row, head 0) + residual.
  LN2 -> h2T (reuses hT slot); FFN pipelined in 512-column slices:
  ffT = relu(W1.T @ h2T + b1') bf16, fc2 rows = ffT.T @ W2 + residual + b2.

All weight reshaping/casting is host-side numpy, shipped as ExternalInputs.
"""

import numpy as np
import ml_dtypes

import concourse.bass as bass
import concourse.mybir as mybir
import concourse.tile as tile
from concourse import bacc
from concourse.bass_utils import run_bass_kernel_spmd

BF16NP = ml_dtypes.bfloat16
BF16 = mybir.dt.bfloat16
F32 = mybir.dt.float32
AF = mybir.ActivationFunctionType
ALU = mybir.AluOpType

P = 128          # partitions
B = 8            # batch -> cores
T = 2048         # sequence length
C = 400          # embed dim
H = 4            # heads
D = 100          # head dim
DFF = 1600       # ffn hidden
NT = T // P      # 16 row tiles
NCC = 4          # c contraction chunks of 128 (last: 16 rows + ones row)
CS = [128, 128, 128, 17]   # chunk heights (incl. bias row in last)
WT = 512         # wide tile for qkv matmuls
NWT = T // WT    # 4
TJ = 512         # t-tile width for transposed attention scores
NTJ = T // TJ    # 4
SUB = TJ // P    # 4 t128 sub-blocks per score tile
FT = 512         # ffn column-slice width
NFT = T // FT    # 4
NFC = (DFF + P - 1) // P  # 13 f-chunks (12x128 + 64)
NEG = -1.0e30

LAST_RESULT = None  # BassKernelResults of the most recent run (for test.py)


def _fchunk(fc):
    return min(P, DFF - fc * P)


def build_block(loop_n=None, phases=("qkv", "attn", "proj", "ffn")):
    nc = bacc.Bacc("TRN2", target_bir_lowering=False, debug=False)

    x_d = nc.dram_tensor("x", [T, C], F32, kind="ExternalInput")
    wq_d = nc.dram_tensor("wqp", [P, H, NCC, P], BF16, kind="ExternalInput")
    wk_d = nc.dram_tensor("wkp", [P, H, NCC, P], BF16, kind="ExternalInput")
    wv_d = nc.dram_tensor("wvp", [P, NCC, C], BF16, kind="ExternalInput")
    wo_d = nc.dram_tensor("wop", [P, H, C], BF16, kind="ExternalInput")
    w1_d = nc.dram_tensor("w1p", [P, NCC, DFF], BF16, kind="ExternalInput")
    w2_d = nc.dram_tensor("w2p", [P, NFC, C], BF16, kind="ExternalInput")
    b2_d = nc.dram_tensor("b2p", [P, C], F32, kind="ExternalInput")
    maskt_d = nc.dram_tensor("masktp", [P, P], BF16, kind="ExternalInput")
    id_d = nc.dram_tensor("identp", [P, P], BF16, kind="ExternalInput")
    out_d = nc.dram_tensor("out", [T, C], F32, kind="ExternalOutput")

    with tile.TileContext(nc) as tc:
        with (
            tc.tile_pool(name="consts", bufs=1) as consts,
            tc.tile_pool(name="persist", bufs=1) as persist,
            tc.tile_pool(name="qk", bufs=2) as qk_pool,
            tc.tile_pool(name="pr", bufs=2) as pr_pool,
            tc.tile_pool(name="fft", bufs=2) as fft_pool,
            tc.tile_pool(name="work", bufs=3) as work,
            tc.tile_pool(name="small", bufs=4) as small,
            tc.tile_pool(name="ps_big", bufs=2, space="PSUM") as ps_big,
            tc.tile_pool(name="ps_av", bufs=2, space="PSUM") as ps_av,
            tc.tile_pool(name="ps_g", bufs=2, space="PSUM") as ps_g,
        ):
            # ---- x into SBUF first (per row-tile, so LN1 starts early);
            # weight/const DMAs are enqueued on the gpsimd queue so their
            # descriptor generation overlaps the x load on sync. ----
            x_sb = persist.tile([P, NT, C], F32, tag="x")
            xr = x_d.rearrange("(n p) c -> p n c", p=P)
            for ti in range(NT):
                nc.sync.dma_start(x_sb[:, ti, :], xr[:, ti, :])

            def cload(tag, dram, shape, dtype, psz=P):
                t_ = consts.tile(shape, dtype, tag=tag)
                nc.gpsimd.dma_start(t_[:psz], dram[:])
                return t_

            id_sb = cload("ident", id_d, [P, P], BF16)
            wq_sb = cload("wq", wq_d, [P, H, NCC, P], BF16)
            wk_sb = cload("wk", wk_d, [P, H, NCC, P], BF16)
            wv_sb = cload("wv", wv_d, [P, NCC, C], BF16)
            maskt_sb = cload("maskt", maskt_d, [P, P], BF16)
            wo_sb = cload("wo", wo_d, [P, H, C], BF16)
            w1_sb = cload("w1", w1_d, [P, NCC, DFF], BF16)
            w2_sb = cload("w2", w2_d, [P, NFC, C], BF16)
            b2_sb = cload("b2", b2_d, [P, C], F32)
            eps_sb = consts.tile([P, 1], F32, tag="eps")
            nc.vector.memset(eps_sb, 1e-5)

            # persistent activations; the constant lanes (v1 ones column for
            # the softmax denominator; the work-tile pads that feed the ao
            # ones row) are written once — no per-iteration re-init.
            hT_sb = persist.tile([P, NCC, T], BF16, tag="hT")
            v1_sb = persist.tile([P, NT, H, D + 2], BF16, tag="v")
            nc.vector.memset(v1_sb[:, :, :, D], 1.0)
            nc.vector.memset(v1_sb[:, :, :, D + 1], 0.0)
            ao_sb = persist.tile([P, H, T], BF16, tag="aoT")
            # explicit rotating work buffers whose pad lanes are constant 1.0
            # (initialized once): cols C.. of hbf become the LN ones row; col
            # D of each arow block becomes the ao/proj-bias ones row.
            hbf_bufs, arow_bufs = [], []
            for i in range(3):
                hb = work.tile([P, 4 * P], BF16, tag=f"hbf{i}")
                nc.vector.memset(hb[:, C:], 1.0)
                hbf_bufs.append(hb)
                ar = work.tile([P, SUB, P], BF16, tag=f"arow{i}")
                nc.vector.memset(ar[:, :, D:], 1.0)
                arow_bufs.append(ar)
            rot = {"hbf": 0, "arow": 0}

            def nextbuf(kind):
                bufs = hbf_bufs if kind == "hbf" else arow_bufs
                t = bufs[rot[kind] % 3]
                rot[kind] += 1
                return t

            def body():

                def layernorm(src3, dstT, tis):
                    """LN (gamma/beta folded into consumers) over row tiles
                    src3[:, ti, :]; bf16 normalized rows + ones col DMA-xbar
                    transposed into dstT[:, cc, ti*P:(ti+1)*P] (c chunks of
                    128; the ones col lands at partition 16 of chunk 3)."""
                    n = len(tis)
                    mv = small.tile([P, n, 2], F32, tag="mv")
                    for k, ti in enumerate(tis):
                        stats = small.tile([P, 6], F32, tag="stats")
                        nc.vector.bn_stats(out=stats, in_=src3[:, ti, :])
                        nc.vector.bn_aggr(out=mv[:, k, :], in_=stats)
                    rstd = small.tile([P, n], F32, tag="rstd")
                    nc.scalar.activation(
                        out=rstd, in_=mv[:, :, 1], func=AF.Sqrt,
                        bias=eps_sb, scale=1.0)
                    nc.vector.reciprocal(out=rstd, in_=rstd)
                    for k, ti in enumerate(tis):
                        hbf = nextbuf("hbf")
                        nc.vector.tensor_scalar(
                            out=hbf[:, :C], in0=src3[:, ti, :],
                            scalar1=mv[:, k, 0:1], scalar2=rstd[:, k:k + 1],
                            op0=ALU.subtract, op1=ALU.mult)
                        nc.sync.dma_start_transpose(
                            dstT[:, :, ti * P:(ti + 1) * P], hbf)

                # ---- LN1 + transpose, in groups of 4 tiles (pipelining) ----
                for g in range(0, NT, 4):
                    layernorm(x_sb, hT_sb, list(range(g, g + 4)))

                # ---- V rows (all heads) + ones column ----
                for ti in range(NT if "qkv" in phases else 0):
                    psv = ps_big.tile([P, 2 * WT], F32, tag="mm")
                    for cc in range(NCC):
                        nc.tensor.matmul(
                            psv[:, :C],
                            lhsT=hT_sb[:CS[cc], cc, ti * P:(ti + 1) * P],
                            rhs=wv_sb[:CS[cc], cc, :],
                            start=(cc == 0), stop=(cc == NCC - 1))
                    nc.scalar.copy(
                        out=v1_sb[:, ti, :, :D],
                        in_=psv[:, :C].rearrange("p (h d) -> p h d", h=H))

                # ---- per-head attention (transposed-score form) ----
                # attn@V is software-pipelined one score-tile behind the
                # scores/exp producer (carried across heads) so independent
                # matmuls hide the ACT exp latency on the in-order PE queue.
                def emit_attnv(pjT, h_, j):
                    pso4 = ps_av.tile([P, SUB, D + 2], F32, tag="av")
                    for jj in range(SUB):
                        ti = SUB * j + jj
                        for si in range(ti + 1):
                            nc.tensor.matmul(
                                pso4[:, jj, :],
                                lhsT=pjT[:, si, jj * P:(jj + 1) * P],
                                rhs=v1_sb[:, si, h_, :],
                                start=(si == 0), stop=(si == ti))
                    rec4 = small.tile([P, SUB], F32, tag="rec")
                    nc.vector.reciprocal(out=rec4, in_=pso4[:, :, D])
                    a4v = nextbuf("arow")
                    nc.vector.tensor_tensor(
                        out=a4v[:, :, :D], in0=pso4[:, :, :D],
                        in1=rec4[:, :, None].to_broadcast((P, SUB, D)),
                        op=ALU.mult)
                    nc.sync.dma_start_transpose(
                        ao_sb[:, h_, j * TJ:(j + 1) * TJ]
                        .rearrange("p (s q) -> p s q", s=SUB),
                        a4v.rearrange("p s q -> p (s q)"))

                def emit_qk_tile(qk_sb, h, tt):
                    sl = slice(tt * WT, (tt + 1) * WT)
                    psqk = ps_big.tile([P, 2 * WT], F32, tag="mm")
                    for w_sb, half in ((wq_sb, 0), (wk_sb, 1)):
                        for cc in range(NCC):
                            nc.tensor.matmul(
                                psqk[:, half * WT:(half + 1) * WT],
                                lhsT=w_sb[:CS[cc], h, cc, :],
                                rhs=hT_sb[:CS[cc], cc, sl],
                                start=(cc == 0), stop=(cc == NCC - 1))
                    nc.vector.tensor_copy(out=qk_sb[:D, :, sl],
                                          in_=psqk[:D, :])

                pend_av = None
                for h in range(H if "qkv" in phases else 0):
                    qk_sb = qk_pool.tile([P, 2, T], BF16, tag="qk")
                    for tt in range(NWT):
                        emit_qk_tile(qk_sb, h, tt)
                    qT = qk_sb[:, 0, :]
                    kT = qk_sb[:, 1, :]

                    for j in range(NTJ if "attn" in phases else 0):
                        pjT = pr_pool.tile([P, NT, TJ], BF16, tag="probsT")
                        # sub-diagonal rows: full width, exp'd in pairs
                        for pr in range(2 * j):
                            pss = ps_big.tile([P, 2 * WT], F32, tag="mm")
                            for q in range(2):
                                nc.tensor.matmul(
                                    pss[:, q * TJ:(q + 1) * TJ],
                                    lhsT=kT[:D, (2 * pr + q) * P:
                                            (2 * pr + q + 1) * P],
                                    rhs=qT[:D, j * TJ:(j + 1) * TJ],
                                    start=True, stop=True)
                            nc.scalar.activation(
                                out=pjT[:, 2 * pr:2 * pr + 2, :], in_=pss,
                                func=AF.Exp)
                        # diagonal block rows: narrowed to live columns,
                        # causal mask added by the PE itself
                        for r in range(SUB):
                            i = SUB * j + r
                            w = TJ - r * P
                            pss = ps_big.tile([P, 2 * WT], F32, tag="mm")
                            nc.tensor.matmul(
                                pss[:, :w], lhsT=kT[:D, i * P:(i + 1) * P],
                                rhs=qT[:D, j * TJ + r * P:(j + 1) * TJ],
                                start=True, stop=False)
                            nc.tensor.matmul(
                                pss[:, :P], lhsT=maskt_sb, rhs=id_sb,
                                start=False, stop=True)
                            nc.scalar.activation(
                                out=pjT[:, i, r * P:], in_=pss[:, :w],
                                func=AF.Exp)
                        if pend_av is not None:
                            emit_attnv(*pend_av)
                        pend_av = (pjT, h, j)

                if pend_av is not None:
                    emit_attnv(*pend_av)

                # ---- output projection + residual (+bo via ones row) ----
                for ti in range(NT if "proj" in phases else 0):
                    psp = ps_g.tile([P, WT], F32, tag="g")
                    for h in range(H):
                        kk = D + 1 if h == 0 else D
                        nc.tensor.matmul(
                            psp[:, :C], lhsT=ao_sb[:kk, h, ti * P:(ti + 1) * P],
                            rhs=wo_sb[:kk, h, :],
                            start=(h == 0), stop=(h == H - 1))
                    nc.vector.tensor_add(out=x_sb[:, ti, :],
                                         in0=x_sb[:, ti, :], in1=psp[:, :C])

                # ---- FFN, pipelined in 512-column slices ----
                outr = out_d.rearrange("(n p) c -> p n c", p=P)
                if "ffn" in phases:
                    h2T = persist.tile([P, NCC, T], BF16, tag="hT")
                    for g in range(0, NT, 4):
                        layernorm(x_sb, h2T, list(range(g, g + 4)))

                    def emit_fc2(ffT, ft):
                        for tl in range(FT // P):
                            ti = ft * (FT // P) + tl
                            psg = ps_g.tile([P, WT], F32, tag="g")
                            for fc in range(NFC):
                                fsz = _fchunk(fc)
                                nc.tensor.matmul(
                                    psg[:, :C],
                                    lhsT=ffT[:fsz, fc, tl * P:(tl + 1) * P],
                                    rhs=w2_sb[:fsz, fc, :],
                                    start=(fc == 0), stop=(fc == NFC - 1))
                            orow = work.tile([P, C], F32, tag="orow")
                            nc.vector.tensor_add(out=orow, in0=psg[:, :C],
                                                 in1=x_sb[:, ti, :])
                            nc.gpsimd.tensor_add(out=orow, in0=orow,
                                                 in1=b2_sb)
                            nc.sync.dma_start(outr[:, ti, :], orow)

                    pend_fc2 = None
                    for ft in range(NFT):
                        sl = slice(ft * FT, (ft + 1) * FT)
                        ffT = fft_pool.tile([P, NFC, FT], BF16, tag="ffT")
                        for fc in range(NFC):
                            fsz = _fchunk(fc)
                            psf = ps_big.tile([P, 2 * WT], F32, tag="mm")
                            for cc in range(NCC):
                                nc.tensor.matmul(
                                    psf[:fsz, :FT],
                                    lhsT=w1_sb[:CS[cc], cc,
                                               fc * P:fc * P + fsz],
                                    rhs=h2T[:CS[cc], cc, sl],
                                    start=(cc == 0), stop=(cc == NCC - 1))
                            if fc % 2 == 0:
                                nc.vector.tensor_scalar_max(
                                    out=ffT[:fsz, fc, :], in0=psf[:fsz, :FT],
                                    scalar1=0.0)
                            else:
                                nc.scalar.activation(
                                    out=ffT[:fsz, fc, :], in_=psf[:fsz, :FT],
                                    func=AF.Relu, bias=0.0, scale=1.0)
                        if pend_fc2 is not None:
                            emit_fc2(*pend_fc2)
                        pend_fc2 = (ffT, ft)
                    emit_fc2(*pend_fc2)
                else:
                    zrow = work.tile([P, C], F32, tag="orow")
                    nc.vector.memset(zrow, 0.0)
                    for ti in range(NT):
                        nc.sync.dma_start(outr[:, ti, :], zrow)

            if loop_n is None:
                body()
            else:
                with tc.For_i(0, loop_n, 1):
                    body()

    nc.finalize()
    return nc


def prep_weights(Wq, Wk, Wv, Wo, bo, W1, b1, W2, b2,
                 ln1_g, ln1_b, ln2_g, ln2_b):
    """Host-side reshape/cast into the layouts the device program expects.
    LayerNorm gains/biases and projection biases are folded in exactly:
      Wq/Wk/Wv rows scaled by ln1_g (Wq also by the 0.1 attn scale); W1 rows
      scaled by ln2_g; each matrix gains a bias contraction row (partition 16
      of c-chunk 3) carrying ln1_b@W (resp. b1 + ln2_b@W1); Wo head 0 gains
      row 100 = bo driven by the ones row of the attn output."""
    f32 = np.float32
    g1 = np.asarray(ln1_g, f32)
    be1 = np.asarray(ln1_b, f32)
    g2 = np.asarray(ln2_g, f32)
    be2 = np.asarray(ln2_b, f32)
    Wq = np.asarray(Wq, f32); Wk = np.asarray(Wk, f32)
    Wv = np.asarray(Wv, f32); Wo = np.asarray(Wo, f32)
    W1 = np.asarray(W1, f32); W2 = np.asarray(W2, f32)
    bq = 0.1 * np.einsum("c,hcd->hd", be1, Wq)   # [H, D]
    bk = np.einsum("c,hcd->hd", be1, Wk)
    bv = np.einsum("c,hcd->hd", be1, Wv)
    Wqs = 0.1 * Wq * g1[None, :, None]
    Wks = Wk * g1[None, :, None]
    Wvs = Wv * g1[None, :, None]
    W1s = W1 * g2[:, None]
    b1f = np.asarray(b1, f32) + be2 @ W1s

    def chunked(Wh, bias):
        """[C, M] + bias [M] -> [128, NCC, M] with rows c-chunked by 128 and
        the bias row at partition 16 of chunk 3."""
        M = Wh.shape[1]
        out = np.zeros((P, NCC, M), BF16NP)
        for cc in range(NCC):
            csz = min(P, C - cc * P)
            out[:csz, cc, :] = Wh[cc * P:cc * P + csz, :].astype(BF16NP)
        out[16, 3, :] = bias.astype(BF16NP)
        return out

    # per-head q/k: [128, H, NCC, 128]
    wqp = np.zeros((P, H, NCC, P), BF16NP)
    wkp = np.zeros((P, H, NCC, P), BF16NP)
    for h in range(H):
        wqp[:, h, :, :D] = chunked(Wqs[h], bq[h])[:, :, :]
        wkp[:, h, :, :D] = chunked(Wks[h], bk[h])[:, :, :]
    # V all heads: [128, NCC, H*D] (+bv bias row)
    wvp = chunked(Wvs.transpose(1, 0, 2).reshape(C, C),
                  bv.reshape(C))
    # Wo: [c_in_head(100)+1, H, C]; row 100 of head 0 = bo
    wop = np.zeros((P, H, C), BF16NP)
    wop[:D] = Wo.reshape(H, D, C).transpose(1, 0, 2).astype(BF16NP)
    wop[D, 0, :] = np.asarray(bo, f32).astype(BF16NP)
    # W1: [128, NCC, DFF] (+b1' bias row)
    w1p = chunked(W1s, b1f)
    # W2: [f_in_chunk(128), fc(13), C], zero-padded
    w2p = np.zeros((P, NFC, C), BF16NP)
    for fc in range(NFC):
        fsz = _fchunk(fc)
        w2p[:fsz, fc, :] = W2[fc * P:fc * P + fsz, :].astype(BF16NP)
    tilep = lambda a: np.tile(np.asarray(a, f32).reshape(1, C), (P, 1)).copy()
    # PE-added causal mask: matmul(lhsT=masktp, rhs=I) adds masktp.T where
    # masktp[t, s] = NEG iff t < s  (strict upper triangle NEG).
    tl_ = np.arange(P)[:, None]
    sl_ = np.arange(P)[None, :]
    masktp = np.where(tl_ >= sl_, 0.0, NEG).astype(BF16NP)
    ident = np.eye(P, dtype=BF16NP)
    return {
        "wqp": wqp, "wkp": wkp, "wvp": wvp, "wop": wop, "w1p": w1p,
        "w2p": w2p, "b2p": tilep(b2),
        "masktp": np.ascontiguousarray(masktp), "identp": ident,
    }


_CACHED_NC = None


def kernel(x, ln1_g, ln1_b, ln2_g, ln2_b, Wq, Wk, Wv, Wo, bo, W1, b1, W2, b2,
           trace=False):
    global _CACHED_NC, LAST_RESULT
    x = np.asarray(x, np.float32)
    assert x.shape == (B, T, C), x.shape
    wmap = prep_weights(Wq, Wk, Wv, Wo, bo, W1, b1, W2, b2,
                        ln1_g, ln1_b, ln2_g, ln2_b)
    if _CACHED_NC is None:
        _CACHED_NC = build_block()
    nc = _CACHED_NC
    in_maps = [dict(wmap, x=np.ascontiguousarray(x[c])) for c in range(B)]
    res = run_bass_kernel_spmd(nc, in_maps, core_ids=list(range(B)),
                               trace=trace)
    LAST_RESULT = res
    out = np.stack([res.results[c]["out"] for c in range(B)])
    return out.astype(np.float32)
